# revision 1
# baseline (speedup 1.0000x reference)
"""Trainium2 Bass kernel for nn_CompSAE (topk_masking, memory-bound).

Math (after host-side folding of the seq_len-1 attention + biases):
    f  = relu(x @ W1 + b1_eff)            # [N, 256],  W1 = W_enc_f
    c  = relu(f @ W2 + b2)                # [N, 128],  W2 = W_enc_c
    bn = relu(c @ Wb + bb)                # [N, 32],   Wb = W_v.T @ W_out.T @ W_bottleneck
    y  = bn @ W_dec + f @ W_res + b_dec   # [N, 2048]

Sharding: pure data-parallel over the token axis N=131072 across 8 cores
(16384 tokens/core). All weights replicated.

The host pre-transposes + fp16-casts x (so the contraction dim lands on SBUF
partitions with plain contiguous DMAs), and all matmuls run in fp16 with fp32
PSUM accumulation (~4e-4 scale-relative error). Activations stay feature-major
(fT/cT/bnT = [feature, token]) so every matmul's stationary operand is either a
natural-layout weight chunk or a feature-major activation; the final decode
matmul then lands token-major, matching the contiguous fp32 output DMA.
b_dec rides along as a ones-row in the bnT stationary (Wdec augmented row).
"""

import os
import numpy as np

N_TOK, D_IN, D_F, D_C, K_BN = 131072, 2048, 256, 128, 32
N_CORES = 8
SHARD = N_TOK // N_CORES          # 16384 tokens per core
TOK = 512                         # supertile tokens
N_SUPER = SHARD // TOK            # 32 supertiles
KC = D_IN // 128                  # 16 contraction chunks for mm1

_CACHE = {}


def _build_nc():
    import concourse.tile as tile
    from concourse import bacc, mybir
    from concourse.bass import ts

    f32 = mybir.dt.float32
    f16 = mybir.dt.float16
    Relu = mybir.ActivationFunctionType.Relu

    nc = bacc.Bacc(None, target_bir_lowering=False)

    xT_d = nc.dram_tensor("xT", [D_IN, SHARD], f16, kind="ExternalInput")
    w1_d = nc.dram_tensor("w1", [KC, 128, D_F], f16, kind="ExternalInput")
    w2_d = nc.dram_tensor("w2", [2, 128, D_C], f16, kind="ExternalInput")
    # wb zero-padded to [128, 128] (cols 32..127 zero) and wdec_aug zero-padded
    # to [128, 2048] (rows 33..127 zero): uniform 128-row stationaries pipeline
    # on the PE; the K=33 form stalled ~250ns per matmul on reconfig.
    wb_d = nc.dram_tensor("wb", [D_C, 128], f16, kind="ExternalInput")
    wres_d = nc.dram_tensor("wres", [2, 128, D_IN], f16, kind="ExternalInput")
    wdec_d = nc.dram_tensor("wdec", [128, D_IN], f16, kind="ExternalInput")
    b1_d = nc.dram_tensor("b1", [128, 2], f32, kind="ExternalInput")
    b2_d = nc.dram_tensor("b2", [128, 1], f32, kind="ExternalInput")
    bb_d = nc.dram_tensor("bb", [K_BN, 1], f32, kind="ExternalInput")
    ones_d = nc.dram_tensor("ones", [1, TOK], f16, kind="ExternalInput")
    y_d = nc.dram_tensor("y", [SHARD, D_IN], f32, kind="ExternalOutput")

    with tile.TileContext(nc) as tc:
        with (
            tc.tile_pool(name="const", bufs=1) as const,
            tc.tile_pool(name="xtp", bufs=3) as xtp,
            tc.tile_pool(name="fp", bufs=2) as fp,
            tc.tile_pool(name="cp", bufs=2) as cp,
            tc.tile_pool(name="bnp", bufs=2) as bnp,
            tc.tile_pool(name="yp", bufs=2) as yp,
            tc.tile_pool(name="fps", bufs=2, space="PSUM") as fps,
            tc.tile_pool(name="sps", bufs=2, space="PSUM") as sps,
            tc.tile_pool(name="yps", bufs=3, space="PSUM") as yps,
        ):
            w1_sb = const.tile([128, KC, D_F], f16)
            nc.sync.dma_start(w1_sb[:], w1_d.rearrange("a p n -> p a n"))
            w2_sb = const.tile([128, 2, D_C], f16)
            nc.sync.dma_start(w2_sb[:], w2_d.rearrange("a p n -> p a n"))
            wb_sb = const.tile([D_C, 128], f16)
            nc.sync.dma_start(wb_sb[:], wb_d[:])
            wres_sb = const.tile([128, 2, D_IN], f16)
            nc.sync.dma_start(wres_sb[:], wres_d.rearrange("a p n -> p a n"))
            wdec_sb = const.tile([128, D_IN], f16)
            nc.sync.dma_start(wdec_sb[:], wdec_d[:])
            b1_sb = const.tile([128, 2], f32)
            nc.sync.dma_start(b1_sb[:], b1_d[:])
            b2_sb = const.tile([128, 1], f32)
            nc.sync.dma_start(b2_sb[:], b2_d[:])
            bb_sb = const.tile([K_BN, 1], f32)
            nc.sync.dma_start(bb_sb[:], bb_d[:])
            ones_sb = const.tile([1, TOK], f16)
            nc.sync.dma_start(ones_sb[:], ones_d[:])

            for t in range(N_SUPER):
                t0 = t * TOK
                xT = xtp.tile([128, KC, TOK], f16)
                nc.sync.dma_start(
                    xT[:], xT_d[:, t0:t0 + TOK].rearrange("(c p) t -> p c t", p=128)
                )

                # mm1: fT[df_chunk m] = sum_c W1[c,:,m*128:+128].T @ xT[c]
                fT = fp.tile([128, 2, TOK], f16)
                for m in range(2):
                    ps = fps.tile([128, TOK], f32)
                    for c in range(KC):
                        nc.tensor.matmul(
                            ps[:], w1_sb[:, c, ts(m, 128)], xT[:, c, :],
                            start=(c == 0), stop=(c == KC - 1),
                        )
                    nc.scalar.activation(fT[:, m, :], ps[:], Relu, bias=b1_sb[:, m:m + 1])

                # mm2: cT = sum_m W2[m].T @ fT[m]
                cps = sps.tile([128, TOK], f32, tag="small")
                for m in range(2):
                    nc.tensor.matmul(
                        cps[:], w2_sb[:, m, :], fT[:, m, :],
                        start=(m == 0), stop=(m == 1),
                    )
                cT = cp.tile([128, TOK], f16)
                nc.scalar.activation(cT[:], cps[:], Relu, bias=b2_sb[:])

                # mm3: bnT = Wb.T @ cT, ones row for b_dec, zero rows 33..127 so
                # the mm4 stationary is a uniform 128-row tile.
                bps = sps.tile([128, TOK], f32, tag="small")
                nc.tensor.matmul(bps[:], wb_sb[:], cT[:])
                bnT = bnp.tile([128, TOK], f16)
                nc.gpsimd.memset(bnT[:], 0.0)
                nc.scalar.activation(bnT[0:K_BN, :], bps[0:K_BN, :], Relu, bias=bb_sb[:])
                nc.vector.tensor_copy(out=bnT[K_BN:K_BN + 1, :], in_=ones_sb[:])

                # mm4: y[tok_half th, 512-chunk n] = fT.T@Wres + bnT_aug.T@Wdec_aug
                y_sb = yp.tile([128, TOK // 128, D_IN], f32)
                for th in range(TOK // 128):
                    for n in range(4):
                        ps = yps.tile([128, 512], f32)
                        nc.tensor.matmul(
                            ps[:], fT[:, 0, ts(th, 128)], wres_sb[:, 0, ts(n, 512)],
                            start=True, stop=False,
                        )
                        nc.tensor.matmul(
                            ps[:], fT[:, 1, ts(th, 128)], wres_sb[:, 1, ts(n, 512)],
                            start=False, stop=False,
                        )
                        nc.tensor.matmul(
                            ps[:], bnT[:, ts(th, 128)], wdec_sb[:, ts(n, 512)],
                            start=False, stop=True,
                        )
                        if (th * 4 + n) % 2 == 0:
                            nc.scalar.copy(out=y_sb[:, th, ts(n, 512)], in_=ps[:])
                        else:
                            nc.vector.tensor_copy(out=y_sb[:, th, ts(n, 512)], in_=ps[:])

                nc.scalar.dma_start(
                    y_d[t0:t0 + TOK, :].rearrange("(a p) d -> p a d", p=128), y_sb[:]
                )

    nc.compile()
    return nc


def _fold_weights(inputs):
    f64 = np.float64
    W1 = np.asarray(inputs["W_enc_f"], np.float32)
    W2 = np.asarray(inputs["W_enc_c"], np.float32)
    W_v = np.asarray(inputs["W_v"], f64)
    b_v = np.asarray(inputs["b_v"], f64)
    W_out = np.asarray(inputs["W_out"], f64)
    b_out = np.asarray(inputs["b_out"], f64)
    W_bn = np.asarray(inputs["W_bottleneck"], f64)
    W_dec = np.asarray(inputs["W_dec"], np.float32)
    b_dec = np.asarray(inputs["b_dec"], np.float32)
    W_res = np.asarray(inputs["W_res"], np.float32)
    b1_eff = (np.asarray(inputs["b_enc_f"], f64)
              - np.asarray(inputs["b_dec"], f64) @ np.asarray(inputs["W_enc_f"], f64))
    Wb = (W_v.T @ W_out.T) @ W_bn                      # [128, 32]
    bb = (b_v @ W_out.T + b_out) @ W_bn                # [32]
    wdec_aug = np.vstack([W_dec, b_dec[None, :]])      # [33, 2048]

    return {
        "w1": np.ascontiguousarray(W1.reshape(KC, 128, D_F).astype(np.float16)),
        "w2": np.ascontiguousarray(W2.reshape(2, 128, D_C).astype(np.float16)),
        "wb": np.ascontiguousarray(
            np.pad(Wb.astype(np.float16), ((0, 0), (0, 128 - K_BN)))),
        "wres": np.ascontiguousarray(W_res.reshape(2, 128, D_IN).astype(np.float16)),
        "wdec": np.ascontiguousarray(
            np.pad(wdec_aug.astype(np.float16), ((0, 128 - K_BN - 1), (0, 0)))),
        "b1": np.ascontiguousarray(b1_eff.astype(np.float32).reshape(2, 128).T),
        "b2": np.ascontiguousarray(np.asarray(inputs["b_enc_c"], np.float32).reshape(128, 1)),
        "bb": np.ascontiguousarray(bb.astype(np.float32).reshape(K_BN, 1)),
        "ones": np.ones((1, TOK), np.float16),
    }


def kernel(**inputs) -> np.ndarray:
    from concourse.bass_utils import run_bass_kernel_spmd

    if "nc" not in _CACHE:
        _CACHE["nc"] = _build_nc()
    nc = _CACHE["nc"]

    x = np.asarray(inputs["acts"], np.float32)
    weights = _fold_weights(inputs)

    in_maps = []
    for i in range(N_CORES):
        xT_i = np.ascontiguousarray(
            x[i * SHARD:(i + 1) * SHARD, :].T.astype(np.float16)
        )
        m = {"xT": xT_i}
        m.update(weights)
        in_maps.append(m)

    trace = bool(os.environ.get("BASS_KERNEL_TRACE"))
    res = run_bass_kernel_spmd(
        nc, in_maps, core_ids=list(range(N_CORES)), trace=trace,
    )
    _CACHE["last_result"] = res
    return np.concatenate([res.results[i]["y"] for i in range(N_CORES)], axis=0)



# revision 2
# speedup vs baseline: 1.0766x; 1.0766x over previous
"""Trainium2 Bass kernel for nn_CompSAE (topk_masking, memory-bound).

Math (after host-side folding of the seq_len-1 attention + biases):
    f  = relu(x @ W1 + b1_eff)            # [N, 256],  W1 = W_enc_f
    c  = relu(f @ W2 + b2)                # [N, 128],  W2 = W_enc_c
    bn = relu(c @ Wb + bb)                # [N, 32],   Wb = W_v.T @ W_out.T @ W_bottleneck
    y  = bn @ W_dec + f @ W_res + b_dec   # [N, 2048]

Sharding: pure data-parallel over the token axis N=131072 across 8 cores
(16384 tokens/core). All weights replicated.

Device computes y0 = bn @ W_dec + f @ W_res in fp16 (fp32 PSUM accumulation);
the host adds b_dec and upcasts to fp32. fp16 output halves the HBM write
volume (the DMA side was co-critical with the PE at fp32).

PE-cycle savers vs the naive decomposition:
  * Wb columns are quadrupled ([wb|wb|wb|wb]) so the single mm3 matmul lands
    bn replicated in all four 32-partition groups at no extra cost.
  * The 16 bn-decode matmuls per supertile (contraction only 32) are packed
    4-way into the PE array via tile_position=(32*th, 0): W_dec is tiled 4x
    across partition groups and each token-chunk's matmul streams through its
    own 32-row strip concurrently (~4 slots instead of 16).

Host pre-transposes + fp16-casts x into a supertile-major blocked layout
[N_SUPER, 128, KC, TOK] so each supertile's input DMA is one contiguous
16KB-per-partition transfer; weights are pre-laid-out partition-major.
"""

import os
import numpy as np

N_TOK, D_IN, D_F, D_C, K_BN = 131072, 2048, 256, 128, 32
N_CORES = 8
SHARD = N_TOK // N_CORES          # 16384 tokens per core
TOK = 512                         # supertile tokens
N_SUPER = SHARD // TOK            # 32 supertiles
KC = D_IN // 128                  # 16 contraction chunks for mm1

_CACHE = {}


def _build_nc():
    import concourse.tile as tile
    from concourse import bacc, mybir
    from concourse.bass import ts

    f32 = mybir.dt.float32
    f16 = mybir.dt.float16
    Relu = mybir.ActivationFunctionType.Relu

    nc = bacc.Bacc(None, target_bir_lowering=False)

    x_d = nc.dram_tensor("xb", [N_SUPER, 128, KC, TOK], f16, kind="ExternalInput")
    w1_d = nc.dram_tensor("w1", [128, KC, D_F], f16, kind="ExternalInput")
    w2_d = nc.dram_tensor("w2", [128, 2, D_C], f16, kind="ExternalInput")
    wbq_d = nc.dram_tensor("wbq", [D_C, 128], f16, kind="ExternalInput")
    wres_d = nc.dram_tensor("wres", [128, 2, D_IN], f16, kind="ExternalInput")
    wdec_d = nc.dram_tensor("wdec", [128, D_IN], f16, kind="ExternalInput")
    b1_d = nc.dram_tensor("b1", [128, 2], f32, kind="ExternalInput")
    b2_d = nc.dram_tensor("b2", [128, 1], f32, kind="ExternalInput")
    bb_d = nc.dram_tensor("bb", [128, 1], f32, kind="ExternalInput")
    y_d = nc.dram_tensor("y", [SHARD, D_IN], f16, kind="ExternalOutput")

    with tile.TileContext(nc) as tc:
        with (
            tc.tile_pool(name="const", bufs=1) as const,
            tc.tile_pool(name="xtp", bufs=3) as xtp,
            tc.tile_pool(name="fp", bufs=2) as fp,
            tc.tile_pool(name="cp", bufs=2) as cp,
            tc.tile_pool(name="bnp", bufs=2) as bnp,
            tc.tile_pool(name="yp", bufs=2) as yp,
            tc.tile_pool(name="fps", bufs=2, space="PSUM") as fps,
            tc.tile_pool(name="sps", bufs=2, space="PSUM") as sps,
            tc.tile_pool(name="yps", bufs=4, space="PSUM") as yps,
        ):
            w1_sb = const.tile([128, KC, D_F], f16)
            nc.sync.dma_start(w1_sb[:], w1_d[:])
            w2_sb = const.tile([128, 2, D_C], f16)
            nc.sync.dma_start(w2_sb[:], w2_d[:])
            wbq_sb = const.tile([D_C, 128], f16)
            nc.sync.dma_start(wbq_sb[:], wbq_d[:])
            wres_sb = const.tile([128, 2, D_IN], f16)
            nc.sync.dma_start(wres_sb[:], wres_d[:])
            wdec_sb = const.tile([128, D_IN], f16)
            nc.sync.dma_start(wdec_sb[:], wdec_d[:])
            b1_sb = const.tile([128, 2], f32)
            nc.sync.dma_start(b1_sb[:], b1_d[:])
            b2_sb = const.tile([128, 1], f32)
            nc.sync.dma_start(b2_sb[:], b2_d[:])
            bb_sb = const.tile([128, 1], f32)
            nc.sync.dma_start(bb_sb[:], bb_d[:])

            for t in range(N_SUPER):
                t0 = t * TOK
                xT = xtp.tile([128, KC, TOK], f16)
                nc.sync.dma_start(xT[:], x_d[t])

                # mm1: fT[df_chunk m] = sum_c W1[:, c, m*128:+128].T @ xT[:, c]
                fT = fp.tile([128, 2, TOK], f16)
                for m in range(2):
                    ps = fps.tile([128, TOK], f32)
                    for c in range(KC):
                        nc.tensor.matmul(
                            ps[:], w1_sb[:, c, ts(m, 128)], xT[:, c, :],
                            start=(c == 0), stop=(c == KC - 1),
                        )
                    nc.scalar.activation(fT[:, m, :], ps[:], Relu, bias=b1_sb[:, m:m + 1])

                # mm2: cT = sum_m W2[m].T @ fT[m]
                cps = sps.tile([128, TOK], f32, tag="small")
                for m in range(2):
                    nc.tensor.matmul(
                        cps[:], w2_sb[:, m, :], fT[:, m, :],
                        start=(m == 0), stop=(m == 1),
                    )
                cT = cp.tile([128, TOK], f16)
                nc.scalar.activation(cT[:], cps[:], Relu, bias=b2_sb[:])

                # mm3: Wb columns are pre-quadrupled, so bn lands replicated in
                # all four 32-partition groups of bnT in one matmul+activation.
                bps = sps.tile([128, TOK], f32, tag="small")
                nc.tensor.matmul(bps[:], wbq_sb[:], cT[:])
                bnT = bnp.tile([128, TOK], f16)
                nc.scalar.activation(bnT[:], bps[:], Relu, bias=bb_sb[:])

                # mm4: y0[tok, n-chunk] = fT.T @ Wres + bnT.T @ Wdec.
                # The four bn matmuls (contraction 32) are issued back-to-back
                # into distinct 32-row PE strips so they execute concurrently.
                y_sb = yp.tile([128, TOK // 128, D_IN], f16)
                for n in range(4):
                    pss = []
                    for th in range(TOK // 128):
                        ps = yps.tile([128, 512], f32)
                        pss.append(ps)
                        nc.tensor.matmul(
                            ps[:], fT[:, 0, ts(th, 128)], wres_sb[:, 0, ts(n, 512)],
                            start=True, stop=False,
                        )
                        nc.tensor.matmul(
                            ps[:], fT[:, 1, ts(th, 128)], wres_sb[:, 1, ts(n, 512)],
                            start=False, stop=False,
                        )
                    for th in range(TOK // 128):
                        nc.tensor.matmul(
                            pss[th][:],
                            bnT[ts(th, 32), ts(th, 128)],
                            wdec_sb[ts(th, 32), ts(n, 512)],
                            start=False, stop=True,
                            tile_position=(32 * th, 0),
                        )
                    for th in range(TOK // 128):
                        if th % 2 == 0:
                            nc.scalar.copy(out=y_sb[:, th, ts(n, 512)], in_=pss[th][:])
                        else:
                            nc.vector.tensor_copy(out=y_sb[:, th, ts(n, 512)], in_=pss[th][:])

                nc.scalar.dma_start(
                    y_d[t0:t0 + TOK, :].rearrange("(a p) d -> p a d", p=128), y_sb[:]
                )

    nc.compile()
    return nc


def _fold_weights(inputs):
    f64 = np.float64
    W1 = np.asarray(inputs["W_enc_f"], np.float32)
    W2 = np.asarray(inputs["W_enc_c"], np.float32)
    W_v = np.asarray(inputs["W_v"], f64)
    b_v = np.asarray(inputs["b_v"], f64)
    W_out = np.asarray(inputs["W_out"], f64)
    b_out = np.asarray(inputs["b_out"], f64)
    W_bn = np.asarray(inputs["W_bottleneck"], f64)
    W_dec = np.asarray(inputs["W_dec"], np.float32)
    W_res = np.asarray(inputs["W_res"], np.float32)
    b1_eff = (np.asarray(inputs["b_enc_f"], f64)
              - np.asarray(inputs["b_dec"], f64) @ np.asarray(inputs["W_enc_f"], f64))
    Wb = (W_v.T @ W_out.T) @ W_bn                      # [128, 32]
    bb = (b_v @ W_out.T + b_out) @ W_bn                # [32]

    return {
        # weights partition-major so every DMA is a straight contiguous copy
        "w1": np.ascontiguousarray(
            W1.reshape(KC, 128, D_F).transpose(1, 0, 2).astype(np.float16)),
        "w2": np.ascontiguousarray(
            W2.reshape(2, 128, D_C).transpose(1, 0, 2).astype(np.float16)),
        "wbq": np.ascontiguousarray(np.tile(Wb.astype(np.float16), (1, 4))),
        "wres": np.ascontiguousarray(
            W_res.reshape(2, 128, D_IN).transpose(1, 0, 2).astype(np.float16)),
        "wdec": np.ascontiguousarray(np.tile(W_dec.astype(np.float16), (4, 1))),
        "b1": np.ascontiguousarray(b1_eff.astype(np.float32).reshape(2, 128).T),
        "b2": np.ascontiguousarray(np.asarray(inputs["b_enc_c"], np.float32).reshape(128, 1)),
        "bb": np.ascontiguousarray(
            np.tile(bb.astype(np.float32), 4).reshape(128, 1)),
    }


def kernel(**inputs) -> np.ndarray:
    from concourse.bass_utils import run_bass_kernel_spmd

    if "nc" not in _CACHE:
        _CACHE["nc"] = _build_nc()
    nc = _CACHE["nc"]

    x = np.asarray(inputs["acts"], np.float32)
    b_dec = np.asarray(inputs["b_dec"], np.float32)
    weights = _fold_weights(inputs)

    in_maps = []
    for i in range(N_CORES):
        xs = x[i * SHARD:(i + 1) * SHARD, :]
        # xb[t, p, c, tok] = x[t*TOK + tok, c*128 + p]
        xb = np.ascontiguousarray(
            xs.reshape(N_SUPER, TOK, KC, 128).transpose(0, 3, 2, 1).astype(np.float16)
        )
        m = {"xb": xb}
        m.update(weights)
        in_maps.append(m)

    trace = bool(os.environ.get("BASS_KERNEL_TRACE"))
    res = run_bass_kernel_spmd(
        nc, in_maps, core_ids=list(range(N_CORES)), trace=trace,
    )
    _CACHE["last_result"] = res
    y0 = np.concatenate([res.results[i]["y"] for i in range(N_CORES)], axis=0)
    return y0.astype(np.float32) + b_dec


# revision 4
# speedup vs baseline: 1.1286x; 1.0483x over previous
"""Trainium2 Bass kernel for nn_CompSAE (topk_masking, memory-bound).

Math (after host-side folding of the seq_len-1 attention + biases):
    f  = relu(x @ W1 + b1_eff)            # [N, 256],  W1 = W_enc_f
    c  = relu(f @ W2 + b2)                # [N, 128],  W2 = W_enc_c
    bn = relu(c @ Wb + bb)                # [N, 32],   Wb = W_v.T @ W_out.T @ W_bottleneck
    y  = bn @ W_dec + f @ W_res + b_dec   # [N, 2048]

Sharding: pure data-parallel over the token axis N=131072 across 8 cores
(16384 tokens/core). All weights replicated.

Device computes y0 = bn @ W_dec + f @ W_res in fp16 (fp32 PSUM accumulation);
the host adds b_dec and upcasts to fp32. fp16 output halves the HBM write
volume (the DMA side was co-critical with the PE at fp32).

PE-cycle savers vs the naive decomposition:
  * Wb columns are quadrupled ([wb|wb|wb|wb]) so the single mm3 matmul lands
    bn replicated in all four 32-partition groups at no extra cost.
  * The 16 bn-decode matmuls per supertile (contraction only 32) are packed
    4-way into the PE array via tile_position=(32*th, 0): W_dec is tiled 4x
    across partition groups and each token-chunk's matmul streams through its
    own 32-row strip concurrently (~4 slots instead of 16).

Host pre-transposes + fp16-casts x into a supertile-major blocked layout
[N_SUPER, 128, KC, TOK] so each supertile's input DMA is one contiguous
16KB-per-partition transfer; weights are pre-laid-out partition-major.
"""

import os
import numpy as np

N_TOK, D_IN, D_F, D_C, K_BN = 131072, 2048, 256, 128, 32
N_CORES = 8
SHARD = N_TOK // N_CORES          # 16384 tokens per core
TOK = 512                         # supertile tokens
N_SUPER = SHARD // TOK            # 32 supertiles
KC = D_IN // 128                  # 16 contraction chunks for mm1

_CACHE = {}


def _build_nc():
    import concourse.tile as tile
    from concourse import bacc, mybir
    from concourse.bass import ts

    f32 = mybir.dt.float32
    f16 = mybir.dt.float16
    Relu = mybir.ActivationFunctionType.Relu

    nc = bacc.Bacc(None, target_bir_lowering=False)

    x_d = nc.dram_tensor("xb", [N_SUPER, 128, KC, TOK], f16, kind="ExternalInput")
    w1_d = nc.dram_tensor("w1", [128, KC, D_F], f16, kind="ExternalInput")
    w2_d = nc.dram_tensor("w2", [128, 2, D_C], f16, kind="ExternalInput")
    wbq_d = nc.dram_tensor("wbq", [D_C, 128], f16, kind="ExternalInput")
    wres_d = nc.dram_tensor("wres", [128, 2, D_IN], f16, kind="ExternalInput")
    wdec_d = nc.dram_tensor("wdec", [128, D_IN], f16, kind="ExternalInput")
    b1_d = nc.dram_tensor("b1", [128, 2], f32, kind="ExternalInput")
    b2_d = nc.dram_tensor("b2", [128, 1], f32, kind="ExternalInput")
    bb_d = nc.dram_tensor("bb", [128, 1], f32, kind="ExternalInput")
    y_d = nc.dram_tensor("y", [SHARD, D_IN], f16, kind="ExternalOutput")

    with tile.TileContext(nc) as tc:
        with (
            tc.tile_pool(name="const", bufs=1) as const,
            tc.tile_pool(name="xtp", bufs=3) as xtp,
            tc.tile_pool(name="fp", bufs=2) as fp,
            tc.tile_pool(name="cp", bufs=2) as cp,
            tc.tile_pool(name="bnp", bufs=2) as bnp,
            tc.tile_pool(name="yp", bufs=2) as yp,
            tc.tile_pool(name="mid", bufs=3, space="PSUM") as mid,
            tc.tile_pool(name="yps", bufs=5, space="PSUM") as yps,
        ):
            # Issue order matters: the sync HWDGE queue drains FIFO, so load
            # only what mm1 needs (w1, b1) before the first x block; the
            # decode-side weights stream in under mm1 of supertile 0.
            w1_sb = const.tile([128, KC, D_F], f16)
            nc.sync.dma_start(w1_sb[:], w1_d[:])
            b1_sb = const.tile([128, 2], f32)
            nc.sync.dma_start(b1_sb[:], b1_d[:])

            xT0 = xtp.tile([128, KC, TOK], f16, tag="xT")
            nc.sync.dma_start(xT0[:], x_d[0])

            w2_sb = const.tile([128, 2, D_C], f16)
            nc.sync.dma_start(w2_sb[:], w2_d[:])
            wbq_sb = const.tile([D_C, 128], f16)
            nc.sync.dma_start(wbq_sb[:], wbq_d[:])
            b2_sb = const.tile([128, 1], f32)
            nc.sync.dma_start(b2_sb[:], b2_d[:])
            bb_sb = const.tile([128, 1], f32)
            nc.sync.dma_start(bb_sb[:], bb_d[:])
            wres_sb = const.tile([128, 2, D_IN], f16)
            nc.sync.dma_start(wres_sb[:], wres_d[:])
            wdec_sb = const.tile([128, D_IN], f16)
            nc.sync.dma_start(wdec_sb[:], wdec_d[:])

            for t in range(N_SUPER):
                t0 = t * TOK
                if t == 0:
                    xT = xT0
                else:
                    xT = xtp.tile([128, KC, TOK], f16, tag="xT")
                    nc.sync.dma_start(xT[:], x_d[t])

                # mm1: fT[df_chunk m] = sum_c W1[:, c, m*128:+128].T @ xT[:, c]
                fT = fp.tile([128, 2, TOK], f16)
                for m in range(2):
                    ps = mid.tile([128, TOK], f32, tag="mid")
                    for c in range(KC):
                        nc.tensor.matmul(
                            ps[:], w1_sb[:, c, ts(m, 128)], xT[:, c, :],
                            start=(c == 0), stop=(c == KC - 1),
                        )
                    nc.scalar.activation(fT[:, m, :], ps[:], Relu, bias=b1_sb[:, m:m + 1])

                # mm2: cT = sum_m W2[m].T @ fT[m]
                cps = mid.tile([128, TOK], f32, tag="mid")
                for m in range(2):
                    nc.tensor.matmul(
                        cps[:], w2_sb[:, m, :], fT[:, m, :],
                        start=(m == 0), stop=(m == 1),
                    )
                cT = cp.tile([128, TOK], f16)
                nc.scalar.activation(cT[:], cps[:], Relu, bias=b2_sb[:])

                # mm3: Wb columns are pre-quadrupled, so bn lands replicated in
                # all four 32-partition groups of bnT in one matmul+activation.
                bps = mid.tile([128, TOK], f32, tag="mid")
                nc.tensor.matmul(bps[:], wbq_sb[:], cT[:])
                bnT = bnp.tile([128, TOK], f16)
                nc.scalar.activation(bnT[:], bps[:], Relu, bias=bb_sb[:])

                # mm4: y0[tok, n-chunk] = fT.T @ Wres + bnT.T @ Wdec.
                # The four bn matmuls (contraction 32) are issued back-to-back
                # into distinct 32-row PE strips so they execute concurrently.
                y_sb = yp.tile([128, TOK // 128, D_IN], f16)
                for n in range(4):
                    pss = []
                    for th in range(TOK // 128):
                        ps = yps.tile([128, 512], f32)
                        pss.append(ps)
                        nc.tensor.matmul(
                            ps[:], fT[:, 0, ts(th, 128)], wres_sb[:, 0, ts(n, 512)],
                            start=True, stop=False,
                        )
                        nc.tensor.matmul(
                            ps[:], fT[:, 1, ts(th, 128)], wres_sb[:, 1, ts(n, 512)],
                            start=False, stop=False,
                        )
                    for th in range(TOK // 128):
                        nc.tensor.matmul(
                            pss[th][:],
                            bnT[ts(th, 32), ts(th, 128)],
                            wdec_sb[ts(th, 32), ts(n, 512)],
                            start=False, stop=True,
                            tile_position=(32 * th, 0),
                        )
                    for th in range(TOK // 128):
                        if th % 2 == 0:
                            nc.scalar.copy(out=y_sb[:, th, ts(n, 512)], in_=pss[th][:])
                        else:
                            nc.vector.tensor_copy(out=y_sb[:, th, ts(n, 512)], in_=pss[th][:])

                nc.scalar.dma_start(
                    y_d[t0:t0 + TOK, :].rearrange("(a p) d -> p a d", p=128), y_sb[:]
                )

    nc.compile()
    return nc


def _fold_weights(inputs):
    f64 = np.float64
    W1 = np.asarray(inputs["W_enc_f"], np.float32)
    W2 = np.asarray(inputs["W_enc_c"], np.float32)
    W_v = np.asarray(inputs["W_v"], f64)
    b_v = np.asarray(inputs["b_v"], f64)
    W_out = np.asarray(inputs["W_out"], f64)
    b_out = np.asarray(inputs["b_out"], f64)
    W_bn = np.asarray(inputs["W_bottleneck"], f64)
    W_dec = np.asarray(inputs["W_dec"], np.float32)
    W_res = np.asarray(inputs["W_res"], np.float32)
    b1_eff = (np.asarray(inputs["b_enc_f"], f64)
              - np.asarray(inputs["b_dec"], f64) @ np.asarray(inputs["W_enc_f"], f64))
    Wb = (W_v.T @ W_out.T) @ W_bn                      # [128, 32]
    bb = (b_v @ W_out.T + b_out) @ W_bn                # [32]

    return {
        # weights partition-major so every DMA is a straight contiguous copy
        "w1": np.ascontiguousarray(
            W1.reshape(KC, 128, D_F).transpose(1, 0, 2).astype(np.float16)),
        "w2": np.ascontiguousarray(
            W2.reshape(2, 128, D_C).transpose(1, 0, 2).astype(np.float16)),
        "wbq": np.ascontiguousarray(np.tile(Wb.astype(np.float16), (1, 4))),
        "wres": np.ascontiguousarray(
            W_res.reshape(2, 128, D_IN).transpose(1, 0, 2).astype(np.float16)),
        "wdec": np.ascontiguousarray(np.tile(W_dec.astype(np.float16), (4, 1))),
        "b1": np.ascontiguousarray(b1_eff.astype(np.float32).reshape(2, 128).T),
        "b2": np.ascontiguousarray(np.asarray(inputs["b_enc_c"], np.float32).reshape(128, 1)),
        "bb": np.ascontiguousarray(
            np.tile(bb.astype(np.float32), 4).reshape(128, 1)),
    }


def kernel(**inputs) -> np.ndarray:
    from concourse.bass_utils import run_bass_kernel_spmd

    if "nc" not in _CACHE:
        _CACHE["nc"] = _build_nc()
    nc = _CACHE["nc"]

    x = np.asarray(inputs["acts"], np.float32)
    b_dec = np.asarray(inputs["b_dec"], np.float32)
    weights = _fold_weights(inputs)

    in_maps = []
    for i in range(N_CORES):
        xs = x[i * SHARD:(i + 1) * SHARD, :]
        # xb[t, p, c, tok] = x[t*TOK + tok, c*128 + p]
        xb = np.ascontiguousarray(
            xs.reshape(N_SUPER, TOK, KC, 128).transpose(0, 3, 2, 1).astype(np.float16)
        )
        m = {"xb": xb}
        m.update(weights)
        in_maps.append(m)

    trace = bool(os.environ.get("BASS_KERNEL_TRACE"))
    res = run_bass_kernel_spmd(
        nc, in_maps, core_ids=list(range(N_CORES)), trace=trace,
    )
    _CACHE["last_result"] = res
    y0 = np.concatenate([res.results[i]["y"] for i in range(N_CORES)], axis=0)
    return y0.astype(np.float32) + b_dec


# revision 5
# speedup vs baseline: 1.1436x; 1.0133x over previous
"""Trainium2 Bass kernel for nn_CompSAE (topk_masking, memory-bound).

Math (after host-side folding of the seq_len-1 attention + biases):
    f  = relu(x @ W1 + b1_eff)            # [N, 256],  W1 = W_enc_f
    c  = relu(f @ W2 + b2)                # [N, 128],  W2 = W_enc_c
    bn = relu(c @ Wb + bb)                # [N, 32],   Wb = W_v.T @ W_out.T @ W_bottleneck
    y  = bn @ W_dec + f @ W_res + b_dec   # [N, 2048]

Sharding: pure data-parallel over the token axis N=131072 across 8 cores
(16384 tokens/core). All weights replicated.

Device computes y0 = bn @ W_dec + f @ W_res in fp16 (fp32 PSUM accumulation);
the host adds b_dec and upcasts to fp32. fp16 output halves the HBM write
volume (the DMA side was co-critical with the PE at fp32).

PE-cycle savers vs the naive decomposition:
  * Wb columns are quadrupled ([wb|wb|wb|wb]) so the single mm3 matmul lands
    bn replicated in all four 32-partition groups at no extra cost.
  * The 16 bn-decode matmuls per supertile (contraction only 32) are packed
    4-way into the PE array via tile_position=(32*th, 0): W_dec is tiled 4x
    across partition groups and each token-chunk's matmul streams through its
    own 32-row strip concurrently (~4 slots instead of 16).

Host pre-transposes + fp16-casts x into a supertile-major blocked layout
[N_SUPER, 128, KC, TOK] so each supertile's input DMA is one contiguous
16KB-per-partition transfer; weights are pre-laid-out partition-major.
"""

import os
import numpy as np

N_TOK, D_IN, D_F, D_C, K_BN = 131072, 2048, 256, 128, 32
N_CORES = 8
SHARD = N_TOK // N_CORES          # 16384 tokens per core
TOK = 512                         # supertile tokens
N_SUPER = SHARD // TOK            # 32 supertiles
KC = D_IN // 128                  # 16 contraction chunks for mm1

_CACHE = {}


def _build_nc():
    import concourse.tile as tile
    from concourse import bacc, mybir
    from concourse.bass import ts

    f32 = mybir.dt.float32
    f16 = mybir.dt.float16
    Relu = mybir.ActivationFunctionType.Relu

    nc = bacc.Bacc(None, target_bir_lowering=False)

    x_d = nc.dram_tensor("xb", [N_SUPER, 128, KC, TOK], f16, kind="ExternalInput")
    w1_d = nc.dram_tensor("w1", [128, KC, D_F], f16, kind="ExternalInput")
    w2_d = nc.dram_tensor("w2", [128, 2, D_C], f16, kind="ExternalInput")
    wbq_d = nc.dram_tensor("wbq", [D_C, 128], f16, kind="ExternalInput")
    wres_d = nc.dram_tensor("wres", [128, 2, D_IN], f16, kind="ExternalInput")
    wdec_d = nc.dram_tensor("wdec", [128, D_IN], f16, kind="ExternalInput")
    b1_d = nc.dram_tensor("b1", [128, 2], f32, kind="ExternalInput")
    b2_d = nc.dram_tensor("b2", [128, 1], f32, kind="ExternalInput")
    bb_d = nc.dram_tensor("bb", [128, 1], f32, kind="ExternalInput")
    y_d = nc.dram_tensor("y", [SHARD, D_IN], f16, kind="ExternalOutput")

    with tile.TileContext(nc) as tc:
        with (
            tc.tile_pool(name="const", bufs=1) as const,
            tc.tile_pool(name="xtp", bufs=3) as xtp,
            tc.tile_pool(name="fp", bufs=2) as fp,
            tc.tile_pool(name="cp", bufs=2) as cp,
            tc.tile_pool(name="bnp", bufs=2) as bnp,
            tc.tile_pool(name="yp", bufs=2) as yp,
            tc.tile_pool(name="mid", bufs=3, space="PSUM") as mid,
            tc.tile_pool(name="yps", bufs=5, space="PSUM") as yps,
        ):
            # Issue order matters: the sync HWDGE queue drains FIFO, so load
            # only what mm1 needs (w1, b1) before the first x block; the
            # decode-side weights stream in under mm1 of supertile 0.
            w1_sb = const.tile([128, KC, D_F], f16)
            nc.sync.dma_start(w1_sb[:], w1_d[:])
            b1_sb = const.tile([128, 2], f32)
            nc.sync.dma_start(b1_sb[:], b1_d[:])

            xT0 = xtp.tile([128, KC, TOK], f16, tag="xT")
            nc.sync.dma_start(xT0[:, 0:KC // 2, :], x_d[0, :, 0:KC // 2, :])
            nc.sync.dma_start(xT0[:, KC // 2:, :], x_d[0, :, KC // 2:, :])

            w2_sb = const.tile([128, 2, D_C], f16)
            nc.sync.dma_start(w2_sb[:], w2_d[:])
            wbq_sb = const.tile([D_C, 128], f16)
            nc.sync.dma_start(wbq_sb[:], wbq_d[:])
            b2_sb = const.tile([128, 1], f32)
            nc.sync.dma_start(b2_sb[:], b2_d[:])
            bb_sb = const.tile([128, 1], f32)
            nc.sync.dma_start(bb_sb[:], bb_d[:])
            wres_sb = const.tile([128, 2, D_IN], f16)
            nc.sync.dma_start(wres_sb[:], wres_d[:])
            wdec_sb = const.tile([128, D_IN], f16)
            nc.sync.dma_start(wdec_sb[:], wdec_d[:])

            # Dummy matmuls gated on the w1 load keep the PE active while the
            # first x block streams in, so HAM is at K=8/8 when mm1(0) starts.
            warm_ps = mid.tile([128, TOK], f32, tag="mid")
            for _ in range(28):
                nc.tensor.matmul(warm_ps[:, 0:256], w1_sb[:, 1, 0:128], w1_sb[:, 0, 0:256])

            def decode_group(fT, bnT, y_sb, n):
                pss = []
                for th in range(TOK // 128):
                    ps = yps.tile([128, 512], f32)
                    pss.append(ps)
                    nc.tensor.matmul(
                        ps[:], fT[:, 0, ts(th, 128)], wres_sb[:, 0, ts(n, 512)],
                        start=True, stop=False,
                    )
                    nc.tensor.matmul(
                        ps[:], fT[:, 1, ts(th, 128)], wres_sb[:, 1, ts(n, 512)],
                        start=False, stop=False,
                    )
                # the four bn matmuls (contraction 32) go to distinct 32-row PE
                # strips back-to-back so they execute concurrently
                for th in range(TOK // 128):
                    nc.tensor.matmul(
                        pss[th][:],
                        bnT[ts(th, 32), ts(th, 128)],
                        wdec_sb[ts(th, 32), ts(n, 512)],
                        start=False, stop=True,
                        tile_position=(32 * th, 0),
                    )
                for th in range(TOK // 128):
                    if th % 2 == 0:
                        nc.scalar.copy(out=y_sb[:, th, ts(n, 512)], in_=pss[th][:])
                    else:
                        nc.vector.tensor_copy(out=y_sb[:, th, ts(n, 512)], in_=pss[th][:])

            def emit_y_dma(t_prev, y_sb, half):
                d0 = half * (D_IN // 2)
                nc.scalar.dma_start(
                    y_d[t_prev * TOK:(t_prev + 1) * TOK, d0:d0 + D_IN // 2]
                    .rearrange("(a p) d -> p a d", p=128),
                    y_sb[:, :, d0:d0 + D_IN // 2],
                )

            # Software pipeline: decode (mm4) of supertile t-1 is interleaved
            # between the encode matmuls of supertile t so the PE never waits
            # on the scalar-engine activations between mm1/mm2/mm3.
            prev = None
            for t in range(N_SUPER):
                if t == 0:
                    xT = xT0
                else:
                    xT = xtp.tile([128, KC, TOK], f16, tag="xT")
                    nc.sync.dma_start(xT[:], x_d[t])

                # mm1: fT[df_chunk m] = sum_c W1[:, c, m*128:+128].T @ xT[:, c]
                fT = fp.tile([128, 2, TOK], f16)
                for m in range(2):
                    ps = mid.tile([128, TOK], f32, tag="mid")
                    for c in range(KC):
                        nc.tensor.matmul(
                            ps[:], w1_sb[:, c, ts(m, 128)], xT[:, c, :],
                            start=(c == 0), stop=(c == KC - 1),
                        )
                    nc.scalar.activation(fT[:, m, :], ps[:], Relu, bias=b1_sb[:, m:m + 1])

                if prev is not None:
                    pt, pfT, pbnT = prev
                    py_sb = yp.tile([128, TOK // 128, D_IN], f16, tag="y")
                    decode_group(pfT, pbnT, py_sb, 0)

                # mm2: cT = sum_m W2[m].T @ fT[m]
                cps = mid.tile([128, TOK], f32, tag="mid")
                for m in range(2):
                    nc.tensor.matmul(
                        cps[:], w2_sb[:, m, :], fT[:, m, :],
                        start=(m == 0), stop=(m == 1),
                    )
                cT = cp.tile([128, TOK], f16)
                nc.scalar.activation(cT[:], cps[:], Relu, bias=b2_sb[:])

                if prev is not None:
                    decode_group(pfT, pbnT, py_sb, 1)

                # mm3: Wb columns are pre-quadrupled, so bn lands replicated in
                # all four 32-partition groups of bnT in one matmul+activation.
                bps = mid.tile([128, TOK], f32, tag="mid")
                nc.tensor.matmul(bps[:], wbq_sb[:], cT[:])
                bnT = bnp.tile([128, TOK], f16)
                nc.scalar.activation(bnT[:], bps[:], Relu, bias=bb_sb[:])

                if prev is not None:
                    decode_group(pfT, pbnT, py_sb, 2)
                    emit_y_dma(pt, py_sb, 0)
                    decode_group(pfT, pbnT, py_sb, 3)
                    emit_y_dma(pt, py_sb, 1)

                prev = (t, fT, bnT)

            pt, pfT, pbnT = prev
            py_sb = yp.tile([128, TOK // 128, D_IN], f16, tag="y")
            decode_group(pfT, pbnT, py_sb, 0)
            decode_group(pfT, pbnT, py_sb, 1)
            emit_y_dma(pt, py_sb, 0)
            decode_group(pfT, pbnT, py_sb, 2)
            decode_group(pfT, pbnT, py_sb, 3)
            emit_y_dma(pt, py_sb, 1)

    nc.compile()
    return nc


def _fold_weights(inputs):
    f64 = np.float64
    W1 = np.asarray(inputs["W_enc_f"], np.float32)
    W2 = np.asarray(inputs["W_enc_c"], np.float32)
    W_v = np.asarray(inputs["W_v"], f64)
    b_v = np.asarray(inputs["b_v"], f64)
    W_out = np.asarray(inputs["W_out"], f64)
    b_out = np.asarray(inputs["b_out"], f64)
    W_bn = np.asarray(inputs["W_bottleneck"], f64)
    W_dec = np.asarray(inputs["W_dec"], np.float32)
    W_res = np.asarray(inputs["W_res"], np.float32)
    b1_eff = (np.asarray(inputs["b_enc_f"], f64)
              - np.asarray(inputs["b_dec"], f64) @ np.asarray(inputs["W_enc_f"], f64))
    Wb = (W_v.T @ W_out.T) @ W_bn                      # [128, 32]
    bb = (b_v @ W_out.T + b_out) @ W_bn                # [32]

    return {
        # weights partition-major so every DMA is a straight contiguous copy
        "w1": np.ascontiguousarray(
            W1.reshape(KC, 128, D_F).transpose(1, 0, 2).astype(np.float16)),
        "w2": np.ascontiguousarray(
            W2.reshape(2, 128, D_C).transpose(1, 0, 2).astype(np.float16)),
        "wbq": np.ascontiguousarray(np.tile(Wb.astype(np.float16), (1, 4))),
        "wres": np.ascontiguousarray(
            W_res.reshape(2, 128, D_IN).transpose(1, 0, 2).astype(np.float16)),
        "wdec": np.ascontiguousarray(np.tile(W_dec.astype(np.float16), (4, 1))),
        "b1": np.ascontiguousarray(b1_eff.astype(np.float32).reshape(2, 128).T),
        "b2": np.ascontiguousarray(np.asarray(inputs["b_enc_c"], np.float32).reshape(128, 1)),
        "bb": np.ascontiguousarray(
            np.tile(bb.astype(np.float32), 4).reshape(128, 1)),
    }


def kernel(**inputs) -> np.ndarray:
    from concourse.bass_utils import run_bass_kernel_spmd

    if "nc" not in _CACHE:
        _CACHE["nc"] = _build_nc()
    nc = _CACHE["nc"]

    x = np.asarray(inputs["acts"], np.float32)
    b_dec = np.asarray(inputs["b_dec"], np.float32)
    weights = _fold_weights(inputs)

    in_maps = []
    for i in range(N_CORES):
        xs = x[i * SHARD:(i + 1) * SHARD, :]
        # xb[t, p, c, tok] = x[t*TOK + tok, c*128 + p]
        xb = np.ascontiguousarray(
            xs.reshape(N_SUPER, TOK, KC, 128).transpose(0, 3, 2, 1).astype(np.float16)
        )
        m = {"xb": xb}
        m.update(weights)
        in_maps.append(m)

    trace = bool(os.environ.get("BASS_KERNEL_TRACE"))
    res = run_bass_kernel_spmd(
        nc, in_maps, core_ids=list(range(N_CORES)), trace=trace,
    )
    _CACHE["last_result"] = res
    y0 = np.concatenate([res.results[i]["y"] for i in range(N_CORES)], axis=0)
    return y0.astype(np.float32) + b_dec


# revision 9
# speedup vs baseline: 1.2302x; 1.0757x over previous
"""Trainium2 Bass kernel for nn_CompSAE (topk_masking, memory-bound).

Math (after host-side folding of the seq_len-1 attention + biases):
    f  = relu(x @ W1 + b1_eff)            # [N, 256],  W1 = W_enc_f
    c  = relu(f @ W2 + b2)                # [N, 128],  W2 = W_enc_c
    bn = relu(c @ Wb + bb)                # [N, 32],   Wb = W_v.T @ W_out.T @ W_bottleneck
    y  = bn @ W_dec + f @ W_res + b_dec   # [N, 2048]

Sharding: pure data-parallel over the token axis N=131072 across 8 cores
(16384 tokens/core). All weights replicated.

Device computes y0 = bn @ W_dec + f @ W_res in fp16 (fp32 PSUM accumulation);
the host adds b_dec and upcasts to fp32. fp16 output halves the HBM write
volume (the DMA side was co-critical with the PE at fp32).

PE-cycle savers vs the naive decomposition:
  * Wb columns are quadrupled ([wb|wb|wb|wb]) so the single mm3 matmul lands
    bn replicated in all four 32-partition groups at no extra cost.
  * The 16 bn-decode matmuls per supertile (contraction only 32) are packed
    4-way into the PE array via tile_position=(32*th, 0): W_dec is tiled 4x
    across partition groups and each token-chunk's matmul streams through its
    own 32-row strip concurrently (~4 slots instead of 16).

Host pre-transposes + fp16-casts x into a supertile-major blocked layout
[N_SUPER, 128, KC, TOK] so each supertile's input DMA is one contiguous
16KB-per-partition transfer; weights are pre-laid-out partition-major.
"""

import os
import numpy as np

N_TOK, D_IN, D_F, D_C, K_BN = 131072, 2048, 256, 128, 32
N_CORES = 8
SHARD = N_TOK // N_CORES          # 16384 tokens per core
TOK = 512                         # supertile tokens
N_SUPER = SHARD // TOK            # 32 supertiles
KC = D_IN // 128                  # 16 contraction chunks for mm1

_CACHE = {}


def _build_nc():
    import concourse.tile as tile
    from concourse import bacc, mybir
    from concourse.bass import ts

    f32 = mybir.dt.float32
    f16 = mybir.dt.float16
    Relu = mybir.ActivationFunctionType.Relu

    nc = bacc.Bacc(None, target_bir_lowering=False)

    x_d = nc.dram_tensor("xb", [N_SUPER, 128, KC, TOK], f16, kind="ExternalInput")
    w1_d = nc.dram_tensor("w1", [128, KC, D_F], f16, kind="ExternalInput")
    w2_d = nc.dram_tensor("w2", [128, 2, D_C], f16, kind="ExternalInput")
    wbq_d = nc.dram_tensor("wbq", [D_C, 128], f16, kind="ExternalInput")
    wres_d = nc.dram_tensor("wres", [128, 2, D_IN], f16, kind="ExternalInput")
    wdec_d = nc.dram_tensor("wdec", [128, D_IN], f16, kind="ExternalInput")
    b1_d = nc.dram_tensor("b1", [128, 2], f32, kind="ExternalInput")
    b2_d = nc.dram_tensor("b2", [128, 1], f32, kind="ExternalInput")
    bb_d = nc.dram_tensor("bb", [128, 1], f32, kind="ExternalInput")
    y_d = nc.dram_tensor("y", [SHARD, D_IN], f16, kind="ExternalOutput")

    with tile.TileContext(nc) as tc:
        with (
            tc.tile_pool(name="const", bufs=1) as const,
            tc.tile_pool(name="xtp", bufs=3) as xtp,
            tc.tile_pool(name="fp", bufs=2) as fp,
            tc.tile_pool(name="cp", bufs=2) as cp,
            tc.tile_pool(name="bnp", bufs=2) as bnp,
            tc.tile_pool(name="yp", bufs=2) as yp,
            tc.tile_pool(name="mid", bufs=2, space="PSUM") as mid,
            tc.tile_pool(name="yps", bufs=6, space="PSUM") as yps,
        ):
            # Issue order matters: the sync HWDGE queue drains FIFO, so load
            # only what mm1 needs (w1, b1) before the first x block; the
            # decode-side weights stream in under mm1 of supertile 0.
            w1_sb = const.tile([128, KC, D_F], f16)
            nc.sync.dma_start(w1_sb[:, 0:2, :], w1_d[:, 0:2, :])
            b1_sb = const.tile([128, 2], f32)
            nc.sync.dma_start(b1_sb[:], b1_d[:])

            xT0 = xtp.tile([128, KC, TOK], f16, tag="xT")
            nc.sync.dma_start(xT0[:, 0:2, :], x_d[0, :, 0:2, :])
            nc.sync.dma_start(w1_sb[:, 2:, :], w1_d[:, 2:, :])
            nc.sync.dma_start(xT0[:, 2:KC // 2, :], x_d[0, :, 2:KC // 2, :])
            nc.sync.dma_start(xT0[:, KC // 2:, :], x_d[0, :, KC // 2:, :])

            w2_sb = const.tile([128, 2, D_C], f16)
            nc.sync.dma_start(w2_sb[:], w2_d[:])
            wbq_sb = const.tile([D_C, 128], f16)
            nc.sync.dma_start(wbq_sb[:], wbq_d[:])
            b2_sb = const.tile([128, 1], f32)
            nc.sync.dma_start(b2_sb[:], b2_d[:])
            bb_sb = const.tile([128, 1], f32)
            nc.sync.dma_start(bb_sb[:], bb_d[:])
            wres_sb = const.tile([128, 2, D_IN], f16)
            nc.sync.dma_start(wres_sb[:], wres_d[:])
            wdec_sb = const.tile([128, D_IN], f16)
            nc.sync.dma_start(wdec_sb[:], wdec_d[:])

            # Dummy matmuls gated on the w1 load keep the PE active while the
            # first x block streams in, so HAM is at K=8/8 when mm1(0) starts.
            warm_ps = mid.tile([128, TOK], f32, tag="mid")
            for _ in range(32):
                nc.tensor.matmul(warm_ps[:, 0:256], w1_sb[:, 1, 0:128], w1_sb[:, 0, 0:256])

            def decode_group(fT, bnT, y_sb, n):
                pss = []
                for th in range(TOK // 128):
                    ps = yps.tile([128, 512], f32)
                    pss.append(ps)
                    nc.tensor.matmul(
                        ps[:], fT[:, 0, ts(th, 128)], wres_sb[:, 0, ts(n, 512)],
                        start=True, stop=False,
                    )
                    nc.tensor.matmul(
                        ps[:], fT[:, 1, ts(th, 128)], wres_sb[:, 1, ts(n, 512)],
                        start=False, stop=False,
                    )
                # the four bn matmuls (contraction 32) go to distinct 32-row PE
                # strips back-to-back so they execute concurrently
                for th in range(TOK // 128):
                    nc.tensor.matmul(
                        pss[th][:],
                        bnT[ts(th, 32), ts(th, 128)],
                        wdec_sb[ts(th, 32), ts(n, 512)],
                        start=False, stop=True,
                        tile_position=(32 * th, 0),
                    )
                for th in range(TOK // 128):
                    if th % 2 == 0:
                        nc.scalar.copy(out=y_sb[:, th, ts(n, 512)], in_=pss[th][:])
                    else:
                        nc.vector.tensor_copy(out=y_sb[:, th, ts(n, 512)], in_=pss[th][:])

            def emit_y_dma(t_prev, y_sb, half):
                d0 = half * (D_IN // 2)
                nc.scalar.dma_start(
                    y_d[t_prev * TOK:(t_prev + 1) * TOK, d0:d0 + D_IN // 2]
                    .rearrange("(a p) d -> p a d", p=128),
                    y_sb[:, :, d0:d0 + D_IN // 2],
                )

            # Software pipeline: decode (mm4) of supertile t-1 is interleaved
            # between the encode matmuls of supertile t so the PE never waits
            # on the scalar-engine activations between mm1/mm2/mm3.
            prev = None
            for t in range(N_SUPER):
                if t == 0:
                    xT = xT0
                else:
                    xT = xtp.tile([128, KC, TOK], f16, tag="xT")
                    nc.sync.dma_start(xT[:], x_d[t])

                # mm1: fT[df_chunk m] = sum_c W1[:, c, m*128:+128].T @ xT[:, c]
                fT = fp.tile([128, 2, TOK], f16)
                for m in range(2):
                    ps = mid.tile([128, TOK], f32, tag="mid")
                    for c in range(KC):
                        nc.tensor.matmul(
                            ps[:], w1_sb[:, c, ts(m, 128)], xT[:, c, :],
                            start=(c == 0), stop=(c == KC - 1),
                        )
                    nc.scalar.activation(fT[:, m, :], ps[:], Relu, bias=b1_sb[:, m:m + 1])

                if prev is not None:
                    pt, pfT, pbnT = prev
                    py_sb = yp.tile([128, TOK // 128, D_IN], f16, tag="y")
                    decode_group(pfT, pbnT, py_sb, 0)

                # mm2: cT = sum_m W2[m].T @ fT[m]
                cps = mid.tile([128, TOK], f32, tag="mid")
                for m in range(2):
                    nc.tensor.matmul(
                        cps[:], w2_sb[:, m, :], fT[:, m, :],
                        start=(m == 0), stop=(m == 1),
                    )
                cT = cp.tile([128, TOK], f16)
                nc.scalar.activation(cT[:], cps[:], Relu, bias=b2_sb[:])

                if prev is not None:
                    decode_group(pfT, pbnT, py_sb, 1)

                # mm3: Wb columns are pre-quadrupled, so bn lands replicated in
                # all four 32-partition groups of bnT in one matmul+activation.
                bps = mid.tile([128, TOK], f32, tag="mid")
                nc.tensor.matmul(bps[:], wbq_sb[:], cT[:])
                bnT = bnp.tile([128, TOK], f16)
                nc.scalar.activation(bnT[:], bps[:], Relu, bias=bb_sb[:])

                if prev is not None:
                    decode_group(pfT, pbnT, py_sb, 2)
                    emit_y_dma(pt, py_sb, 0)
                    decode_group(pfT, pbnT, py_sb, 3)
                    emit_y_dma(pt, py_sb, 1)

                prev = (t, fT, bnT)

            # epilogue: decode the last supertile, draining output per quarter
            # so the final DMA isn't serialized behind all four groups
            pt, pfT, pbnT = prev
            py_sb = yp.tile([128, TOK // 128, D_IN], f16, tag="y")
            for n in range(4):
                decode_group(pfT, pbnT, py_sb, n)
                nc.scalar.dma_start(
                    y_d[pt * TOK:(pt + 1) * TOK, n * 512:(n + 1) * 512]
                    .rearrange("(a p) d -> p a d", p=128),
                    py_sb[:, :, ts(n, 512)],
                )

    nc.compile()
    return nc


def _fold_weights(inputs):
    f64 = np.float64
    W1 = np.asarray(inputs["W_enc_f"], np.float32)
    W2 = np.asarray(inputs["W_enc_c"], np.float32)
    W_v = np.asarray(inputs["W_v"], f64)
    b_v = np.asarray(inputs["b_v"], f64)
    W_out = np.asarray(inputs["W_out"], f64)
    b_out = np.asarray(inputs["b_out"], f64)
    W_bn = np.asarray(inputs["W_bottleneck"], f64)
    W_dec = np.asarray(inputs["W_dec"], np.float32)
    W_res = np.asarray(inputs["W_res"], np.float32)
    b1_eff = (np.asarray(inputs["b_enc_f"], f64)
              - np.asarray(inputs["b_dec"], f64) @ np.asarray(inputs["W_enc_f"], f64))
    Wb = (W_v.T @ W_out.T) @ W_bn                      # [128, 32]
    bb = (b_v @ W_out.T + b_out) @ W_bn                # [32]

    return {
        # weights partition-major so every DMA is a straight contiguous copy
        "w1": np.ascontiguousarray(
            W1.reshape(KC, 128, D_F).transpose(1, 0, 2).astype(np.float16)),
        "w2": np.ascontiguousarray(
            W2.reshape(2, 128, D_C).transpose(1, 0, 2).astype(np.float16)),
        "wbq": np.ascontiguousarray(np.tile(Wb.astype(np.float16), (1, 4))),
        "wres": np.ascontiguousarray(
            W_res.reshape(2, 128, D_IN).transpose(1, 0, 2).astype(np.float16)),
        "wdec": np.ascontiguousarray(np.tile(W_dec.astype(np.float16), (4, 1))),
        "b1": np.ascontiguousarray(b1_eff.astype(np.float32).reshape(2, 128).T),
        "b2": np.ascontiguousarray(np.asarray(inputs["b_enc_c"], np.float32).reshape(128, 1)),
        "bb": np.ascontiguousarray(
            np.tile(bb.astype(np.float32), 4).reshape(128, 1)),
    }


def kernel(**inputs) -> np.ndarray:
    from concourse.bass_utils import run_bass_kernel_spmd

    if "nc" not in _CACHE:
        _CACHE["nc"] = _build_nc()
    nc = _CACHE["nc"]

    x = np.asarray(inputs["acts"], np.float32)
    b_dec = np.asarray(inputs["b_dec"], np.float32)
    weights = _fold_weights(inputs)

    in_maps = []
    for i in range(N_CORES):
        xs = x[i * SHARD:(i + 1) * SHARD, :]
        # xb[t, p, c, tok] = x[t*TOK + tok, c*128 + p]
        xb = np.ascontiguousarray(
            xs.reshape(N_SUPER, TOK, KC, 128).transpose(0, 3, 2, 1).astype(np.float16)
        )
        m = {"xb": xb}
        m.update(weights)
        in_maps.append(m)

    trace = bool(os.environ.get("BASS_KERNEL_TRACE"))
    res = run_bass_kernel_spmd(
        nc, in_maps, core_ids=list(range(N_CORES)), trace=trace,
    )
    _CACHE["last_result"] = res
    y0 = np.concatenate([res.results[i]["y"] for i in range(N_CORES)], axis=0)
    return y0.astype(np.float32) + b_dec
